# revision 1
# baseline (speedup 1.0000x reference)
"""Trainium2 Bass kernel for CachedEHREmbeddings (embedding_lookup).

Strategy (data-parallel over batch):
  - B=32 batch rows -> 4 rows per core x 8 cores; 8192 tokens/core, 64
    tiles of 128 tokens.
  - word / order embeddings: indirect-DMA row gathers from HBM.
  - type / seg embeddings: one-hot matmul on TensorE (tables are tiny;
    avoids two more full gather passes over HBM).
  - time/age sinusoidal features computed on-chip (DVE + ScalarE Sin).
  - fused = [word | sin(time) | sin(age) | 1 | type_rep | seg_rep] is
    transposed on PE, then matmul'd against lin_W chunks (K=833 incl.
    the bias row) accumulating in PSUM; tanh on ScalarE; LayerNorm via
    bn_stats/bn_aggr on DVE.
"""

import sys

for _p in ("/opt/trn_rl_repo",):
    if _p not in sys.path:
        sys.path.insert(0, _p)

import numpy as np

import concourse.bass as bass
import concourse.bacc as bacc
import concourse.tile as tile
from concourse import mybir
from concourse.bass import IndirectOffsetOnAxis
from concourse.bass_utils import run_bass_kernel_spmd

# Problem constants (hardcoded per contract)
V, H, T = 32000, 768, 32
TYPES, MAX_VISITS, SEGS = 9, 512, 3
B, S = 32, 2048
EPS = 1e-12
N_CORES = 8
B_PER = B // N_CORES            # 4 batch rows per core
TOK = B_PER * S                 # 8192 tokens per core
P = 128
NTILES = TOK // P               # 64

F32 = mybir.dt.float32
F32R = mybir.dt.float32r
I32 = mybir.dt.int32

# fused feature layout
C_WORD = 0                      # [0:768]   word embedding (gathered)
C_SIN = H                       # [768:832] time(32) | age(32) sin features
C_ONE = H + 2 * T               # [832]     constant 1.0 (bias row of lin)
C_TYPE = C_ONE + 1              # [833:842] type id replicated x9
C_SEG = C_TYPE + TYPES          # [842:845] seg id replicated x3
FUSED_W = C_SEG + SEGS          # 845
K_MAIN = C_ONE + 1              # 833 contraction dims for the main matmul

# transposed layout: chunks c0..c5 (word), c6a = [768:833] (sin+one, 65 wide),
# c6b = [833:845] (type+seg, 12 wide)
W6A = K_MAIN - 768              # 65
W6B = TYPES + SEGS              # 12

MM_DT = F32R                    # matmul input dtype view (f32r = full PE rate)


def _bcast_rows(ap, p=P):
    """Partition-broadcast a [n]-shaped DRAM AP to [p, n] (stride-0 rows)."""
    return bass.AP(tensor=ap.tensor, offset=ap.offset, ap=[[0, p]] + list(ap.ap))


def build_nc(apply_gb: bool):
    nc = bacc.Bacc("TRN2", target_bir_lowering=False, debug=False,
                   num_devices=N_CORES)

    meta_d = nc.declare_dram_parameter("meta", [TOK, 8], I32, isOutput=False)
    w_word_d = nc.declare_dram_parameter("W_word", [V, H], F32, isOutput=False)
    w_order_d = nc.declare_dram_parameter("W_order", [MAX_VISITS, H], F32, isOutput=False)
    w_ts_d = nc.declare_dram_parameter("W_ts", [TYPES + SEGS, H], F32, isOutput=False)
    lin_w_d = nc.declare_dram_parameter("lin_w", [H + 2 * T, H], F32, isOutput=False)
    lin_b_d = nc.declare_dram_parameter("lin_b", [H], F32, isOutput=False)
    tw_d = nc.declare_dram_parameter("time_w", [1, T], F32, isOutput=False)
    tphi_d = nc.declare_dram_parameter("time_phi", [1, T], F32, isOutput=False)
    aw_d = nc.declare_dram_parameter("age_w", [1, T], F32, isOutput=False)
    aphi_d = nc.declare_dram_parameter("age_phi", [1, T], F32, isOutput=False)
    iota_d = nc.declare_dram_parameter("iota12", [W6B, 1], F32, isOutput=False)
    ident_d = nc.declare_dram_parameter("ident", [P, P], F32, isOutput=False)
    if apply_gb:
        ln_g_d = nc.declare_dram_parameter("ln_g", [H], F32, isOutput=False)
        ln_b_d = nc.declare_dram_parameter("ln_beta", [H], F32, isOutput=False)
    out_d = nc.declare_dram_parameter("out", [TOK, H], F32, isOutput=True)

    with tile.TileContext(nc) as tc:
        with (
            tc.tile_pool(name="singles", bufs=1) as singles,
            tc.tile_pool(name="mp", bufs=4) as mp,
            tc.tile_pool(name="fp", bufs=3) as fp,
            tc.tile_pool(name="ftp", bufs=3) as ftp,
            tc.tile_pool(name="ordp", bufs=3) as ordp,
            tc.tile_pool(name="embp", bufs=3) as embp,
            tc.tile_pool(name="outp", bufs=3) as outp,
            tc.tile_pool(name="sp", bufs=4) as sp,
            tc.tile_pool(name="pst", bufs=2, space="PSUM") as pst,
            tc.tile_pool(name="psm", bufs=2, space="PSUM") as psm,
            tc.tile_pool(name="pso", bufs=1, space="PSUM") as pso,
        ):
            # ---- constants ----
            lw = []
            for c in range(6):
                stg = singles.tile([P, H], F32, tag=f"stg{c}")
                nc.sync.dma_start(out=stg[:], in_=lin_w_d[c * P:(c + 1) * P, :])
                t = singles.tile([P, H], MM_DT, tag=f"lw{c}")
                nc.vector.tensor_copy(out=t[:], in_=stg[:])
                lw.append(t)
            stg = singles.tile([W6A, H], F32, tag="stg6")
            nc.sync.dma_start(out=stg[0:64, :], in_=lin_w_d[768:832, :])
            nc.sync.dma_start(out=stg[64:65, :], in_=lin_b_d[None, :])
            lin7 = singles.tile([W6A, H], MM_DT, tag="lin7")
            nc.vector.tensor_copy(out=lin7[:], in_=stg[:])
            stg = singles.tile([W6B, H], F32, tag="stg7")
            nc.sync.dma_start(out=stg[:], in_=w_ts_d[:])
            wts = singles.tile([W6B, H], MM_DT, tag="wts")
            nc.vector.tensor_copy(out=wts[:], in_=stg[:])
            wb = singles.tile([P, 2 * T], F32, tag="wb")
            nc.sync.dma_start(out=wb[:, 0:T], in_=_bcast_rows(tw_d[0]))
            nc.sync.dma_start(out=wb[:, T:2 * T], in_=_bcast_rows(aw_d[0]))
            phib = singles.tile([P, 2 * T], F32, tag="phib")
            nc.sync.dma_start(out=phib[:, 0:T], in_=_bcast_rows(tphi_d[0]))
            nc.sync.dma_start(out=phib[:, T:2 * T], in_=_bcast_rows(aphi_d[0]))
            iota12 = singles.tile([W6B, 1], F32, tag="iota12")
            nc.sync.dma_start(out=iota12[:], in_=iota_d[:])
            ident = singles.tile([P, P], F32, tag="ident")
            nc.sync.dma_start(out=ident[:], in_=ident_d[:])
            eps_sb = singles.tile([P, 1], F32, tag="eps")
            nc.vector.memset(eps_sb[:], EPS)
            if apply_gb:
                g_sb = singles.tile([P, H], F32, tag="g")
                nc.sync.dma_start(out=g_sb[:], in_=_bcast_rows(ln_g_d[:]))
                b_sb = singles.tile([P, H], F32, tag="b")
                nc.sync.dma_start(out=b_sb[:], in_=_bcast_rows(ln_b_d[:]))

            # ---- per-tile loop ----
            for i in range(NTILES):
                r0 = i * P
                meta = mp.tile([P, 8], I32, tag="meta")
                nc.sync.dma_start(out=meta[:], in_=meta_d[r0:r0 + P, :])

                fused = fp.tile([P, FUSED_W], F32, tag="fused")
                # word gather -> fused[:, 0:768]
                nc.gpsimd.indirect_dma_start(
                    out=fused[:, C_WORD:C_WORD + H],
                    out_offset=None,
                    in_=w_word_d[:, :],
                    in_offset=IndirectOffsetOnAxis(ap=meta[:, 0:1], axis=0),
                )
                # dt = ts - ts_prev
                dt = sp.tile([P, 1], F32, tag="dt")
                nc.vector.tensor_tensor(
                    out=dt[:],
                    in0=meta[:, 4:5].bitcast(F32),
                    in1=meta[:, 5:6].bitcast(F32),
                    op=mybir.AluOpType.subtract,
                )
                # sin features: sin(dt*w + phi), sin(age*w + phi)
                nc.vector.tensor_scalar(
                    out=fused[:, C_SIN:C_SIN + T], in0=wb[:, 0:T],
                    scalar1=dt[:], scalar2=None, op0=mybir.AluOpType.mult,
                )
                nc.vector.tensor_scalar(
                    out=fused[:, C_SIN + T:C_SIN + 2 * T], in0=wb[:, T:2 * T],
                    scalar1=meta[:, 6:7].bitcast(F32), scalar2=None,
                    op0=mybir.AluOpType.mult,
                )
                nc.vector.tensor_add(
                    out=fused[:, C_SIN:C_SIN + 2 * T],
                    in0=fused[:, C_SIN:C_SIN + 2 * T], in1=phib[:],
                )
                nc.scalar.activation(
                    out=fused[:, C_SIN:C_SIN + 2 * T],
                    in_=fused[:, C_SIN:C_SIN + 2 * T],
                    func=mybir.ActivationFunctionType.Sin,
                )
                # constant-1 column (bias row of lin), replicated type/seg ids
                nc.vector.memset(fused[:, C_ONE:C_ONE + 1], 1.0)
                nc.vector.tensor_copy(
                    out=fused[:, C_TYPE:C_TYPE + TYPES],
                    in_=meta[:, 1:2].to_broadcast([P, TYPES]),
                )
                nc.vector.tensor_copy(
                    out=fused[:, C_SEG:C_SEG + SEGS],
                    in_=meta[:, 3:4].to_broadcast([P, SEGS]),
                )

                # ---- transpose fused -> fusedT ----
                tp1 = pst.tile([P, 512], F32, tag="tp", space="PSUM")
                for c in range(4):
                    nc.tensor.transpose(
                        out=tp1[:, c * P:(c + 1) * P],
                        in_=fused[:, c * P:(c + 1) * P], identity=ident[:],
                    )
                fusedT = ftp.tile([P, 1024], F32, tag="fusedT")
                nc.scalar.copy(out=fusedT[:, 0:512].bitcast(MM_DT), in_=tp1[:])
                tp2 = pst.tile([P, 512], F32, tag="tp", space="PSUM")
                nc.tensor.transpose(out=tp2[:, 0:P], in_=fused[:, 512:640], identity=ident[:])
                nc.tensor.transpose(out=tp2[:, P:2 * P], in_=fused[:, 640:768], identity=ident[:])
                nc.tensor.transpose(out=tp2[0:W6A, 2 * P:3 * P], in_=fused[:, 768:768 + W6A], identity=ident[:])
                nc.tensor.transpose(out=tp2[0:W6B, 3 * P:4 * P], in_=fused[:, C_TYPE:FUSED_W], identity=ident[:])
                nc.scalar.copy(out=fusedT[:, 512:896].bitcast(MM_DT), in_=tp2[:, 0:384])
                nc.scalar.copy(out=fusedT[:, 896:1024].bitcast(MM_DT), in_=tp2[:, 384:512])

                # ---- main matmul: mm = fusedT.T @ lin_W (+bias row) ----
                mm = psm.tile([P, H], F32, tag="mm", space="PSUM")
                for c in range(6):
                    lhsT = fusedT[:, c * P:(c + 1) * P].bitcast(MM_DT)
                    for n0, n1 in ((0, 512), (512, 768)):
                        nc.tensor.matmul(
                            out=mm[:, n0:n1], lhsT=lhsT,
                            rhs=lw[c][:, n0:n1],
                            start=(c == 0), stop=False,
                        )
                lhsT7 = fusedT[0:W6A, 768:896].bitcast(MM_DT)
                for n0, n1 in ((0, 512), (512, 768)):
                    nc.tensor.matmul(
                        out=mm[:, n0:n1], lhsT=lhsT7,
                        rhs=lin7[:, n0:n1],
                        start=False, stop=True,
                    )

                # ---- one-hot type/seg matmul ----
                oh = sp.tile([W6B, P], F32, tag="oh")
                nc.vector.tensor_scalar(
                    out=oh[:].bitcast(MM_DT), in0=fusedT[0:W6B, 896:1024],
                    scalar1=iota12[:], scalar2=None,
                    op0=mybir.AluOpType.is_equal,
                )
                ohp = pso.tile([P, H], F32, tag="ohp", space="PSUM")
                for n0, n1 in ((0, 512), (512, 768)):
                    nc.tensor.matmul(
                        out=ohp[:, n0:n1], lhsT=oh[:].bitcast(MM_DT),
                        rhs=wts[:, n0:n1],
                        start=True, stop=True,
                    )

                # ---- tanh + adds ----
                emb = embp.tile([P, H], F32, tag="emb")
                nc.scalar.activation(
                    out=emb[:], in_=mm[:], func=mybir.ActivationFunctionType.Tanh,
                )
                ordt = ordp.tile([P, H], F32, tag="ordt")
                nc.gpsimd.indirect_dma_start(
                    out=ordt[:], out_offset=None,
                    in_=w_order_d[:, :],
                    in_offset=IndirectOffsetOnAxis(ap=meta[:, 2:3], axis=0),
                )
                nc.vector.tensor_add(out=emb[:], in0=emb[:], in1=ohp[:])
                nc.vector.tensor_add(out=emb[:], in0=emb[:], in1=ordt[:])

                # ---- LayerNorm ----
                stats = sp.tile([P, 3, 6], F32, tag="stats")
                for g in range(3):
                    nc.vector.bn_stats(out=stats[:, g, :], in_=emb[:, g * 256:(g + 1) * 256])
                mv = sp.tile([P, 2], F32, tag="mv")
                nc.vector.bn_aggr(out=mv[:], in_=stats[:])
                sd = sp.tile([P, 1], F32, tag="sd")
                nc.scalar.activation(
                    out=sd[:], in_=mv[:, 1:2],
                    func=mybir.ActivationFunctionType.Sqrt, bias=eps_sb[:],
                )
                rstd = sp.tile([P, 1], F32, tag="rstd")
                nc.vector.reciprocal(out=rstd[:], in_=sd[:])

                outt = outp.tile([P, H], F32, tag="outt")
                nc.vector.tensor_scalar(
                    out=outt[:], in0=emb[:],
                    scalar1=mv[:, 0:1], scalar2=rstd[:],
                    op0=mybir.AluOpType.subtract, op1=mybir.AluOpType.mult,
                )
                if apply_gb:
                    nc.vector.tensor_mul(out=outt[:], in0=outt[:], in1=g_sb[:])
                    nc.vector.tensor_add(out=outt[:], in0=outt[:], in1=b_sb[:])

                nc.sync.dma_start(out=out_d[r0:r0 + P, :], in_=outt[:])

    nc.finalize()
    return nc


def _prepare(inputs):
    ids = np.ascontiguousarray(np.asarray(inputs["input_ids"], dtype=np.int32))
    typ = np.ascontiguousarray(np.asarray(inputs["type_ids"], dtype=np.int32))
    order = np.ascontiguousarray(np.asarray(inputs["visit_orders"], dtype=np.int32))
    seg = np.ascontiguousarray(np.asarray(inputs["visit_segments"], dtype=np.int32))
    ts = np.ascontiguousarray(np.asarray(inputs["time_stamps"], dtype=np.float32))
    ages = np.ascontiguousarray(np.asarray(inputs["ages"], dtype=np.float32))

    # halo: ts_prev[b, 0] = ts[b, 0] so dt[b, 0] == 0 (matches reference)
    ts_prev = np.concatenate([ts[:, :1], ts[:, :-1]], axis=1)

    meta = np.zeros((B, S, 8), dtype=np.int32)
    meta[..., 0] = ids
    meta[..., 1] = typ
    meta[..., 2] = order
    meta[..., 3] = seg
    meta[..., 4] = ts.view(np.int32)
    meta[..., 5] = ts_prev.view(np.int32)
    meta[..., 6] = ages.view(np.int32)

    f32c = lambda x: np.ascontiguousarray(np.asarray(x, dtype=np.float32))
    w_type = f32c(inputs["W_type"])
    w_seg = f32c(inputs["W_seg"])
    common = dict(
        W_word=f32c(inputs["W_word"]),
        W_order=f32c(inputs["W_order"]),
        W_ts=np.ascontiguousarray(np.concatenate([w_type, w_seg], axis=0)),
        lin_w=f32c(inputs["lin_W"]),
        lin_b=f32c(inputs["lin_b"]),
        time_w=f32c(inputs["time_w"]),
        time_phi=f32c(inputs["time_phi"]),
        age_w=f32c(inputs["age_w"]),
        age_phi=f32c(inputs["age_phi"]),
        iota12=np.array([[i] for i in list(range(TYPES)) + list(range(SEGS))],
                        dtype=np.float32),
        ident=np.eye(P, dtype=np.float32),
    )

    ln_g = f32c(inputs["ln_g"])
    ln_beta = f32c(inputs["ln_beta"])
    apply_gb = not (np.all(ln_g == 1.0) and np.all(ln_beta == 0.0))
    if apply_gb:
        common["ln_g"] = ln_g
        common["ln_beta"] = ln_beta

    in_maps = []
    for k in range(N_CORES):
        m = dict(common)
        m["meta"] = np.ascontiguousarray(
            meta[k * B_PER:(k + 1) * B_PER].reshape(TOK, 8))
        in_maps.append(m)
    return in_maps, apply_gb


def run(inputs, trace=False):
    in_maps, apply_gb = _prepare(inputs)
    nc = build_nc(apply_gb)
    res = run_bass_kernel_spmd(nc, in_maps, list(range(N_CORES)), trace=trace)
    shards = [res.results[k]["out"].reshape(B_PER, S, H) for k in range(N_CORES)]
    out = np.concatenate(shards, axis=0)
    return out, res


def kernel(**inputs) -> np.ndarray:
    out, _ = run(inputs, trace=False)
    return out



# revision 34
# speedup vs baseline: 1.6587x; 1.6587x over previous
"""Trainium2 Bass kernel for CachedEHREmbeddings (embedding_lookup).

Strategy (data-parallel over batch, 4 batch rows -> 8192 tokens per core):
  - word embeddings: per-tile indirect-DMA row gathers from an fp8e4 copy
    of W_word (values pre-scaled x16).  Gathered rows are transposed on
    the PE as uint16 PAIRS, which lands features in exactly the
    interleaved layout the fp8 DoubleRow matmul wants (2 K-tiles per
    pass, 2x PE throughput).
  - order/type/seg: fused on host into one bf16 table
    W_comb[o*27+t*3+s]; added into the post-tanh tile by an indirect
    DMA with compute_op=add (CCE accumulate) -- no extra DVE pass.
  - time/age sinusoidal features on-chip; w/phi are pre-divided by 2pi,
    wrapped into [-0.5,0.5] turns via the f32 magic-number round, and
    Sin runs with scale=2pi (exact range reduction, sim-clean).
  - main matmul: 3 fp8 DoubleRow chunks + 1 f32r sin chunk (includes the
    bias row, all x256) accumulate in PSUM; tanh on ScalarE with
    scale=1/256 -> bf16.
  - LayerNorm: DVE bn_stats/bn_aggr per tile; Sqrt batched per 16-tile
    group (keeps the ScalarE activation-table on the sin/tanh set);
    normalize alternates DVE/ScalarE to balance engines; bf16 output
    (host upcasts to f32).
"""

import sys

for _p in ("/opt/trn_rl_repo",):
    if _p not in sys.path:
        sys.path.insert(0, _p)

import ml_dtypes
import numpy as np

import concourse.bass as bass
import concourse.bacc as bacc
import concourse.tile as tile
from concourse import mybir
from concourse.bass import IndirectOffsetOnAxis
from concourse.bass_utils import run_bass_kernel_spmd

# Problem constants (hardcoded per contract)
V, H, T = 32000, 768, 32
TYPES, MAX_VISITS, SEGS = 9, 512, 3
COMB = MAX_VISITS * TYPES * SEGS        # 13824 fused order/type/seg rows
B, S = 32, 2048
EPS = 1e-12
N_CORES = 8
B_PER = B // N_CORES                    # 4 batch rows per core
TOK = B_PER * S                         # 8192 tokens per core
P = 128
NTILES = TOK // P                       # 64
GTILES = 16                             # tiles per layernorm/sqrt group
NG = NTILES // GTILES

WSCALE = 16.0                           # fp8 scale for W_word and lin_W word part
MMSCALE = WSCALE * WSCALE               # total matmul scale (word path)

F32 = mybir.dt.float32
F32R = mybir.dt.float32r
BF16 = mybir.dt.bfloat16
F8 = mybir.dt.float8e4
U16 = mybir.dt.uint16
I32 = mybir.dt.int32

BF16_NP = ml_dtypes.bfloat16
F8_NP = ml_dtypes.float8_e4m3


def _bcast_rows(ap, p=P):
    """Partition-broadcast a [n]-shaped DRAM AP to [p, n] (stride-0 rows)."""
    return bass.AP(tensor=ap.tensor, offset=ap.offset, ap=[[0, p]] + list(ap.ap))


def _pair_lhsT(wt_f8, q):
    """lhsT AP [128, 2, 128] for DoubleRow chunk-pair q of the transposed
    word tile.  FP8 PE transposes write with element step 2; chunks 2q and
    2q+1 are interleaved byte-wise in bytes [512q : 512q+512) (chunk 2q at
    even bytes, 2q+1 at odd).  Partition p of chunk c holds feature
    128c+p; k-tile j of pair q is chunk 2q+j."""
    return bass.AP(
        tensor=wt_f8.tensor, offset=wt_f8.offset + 512 * q,
        ap=[list(wt_f8.ap[0]), [256, 2], [2, P]],
    )


def build_nc(apply_gb: bool):
    nc = bacc.Bacc("TRN2", target_bir_lowering=False, debug=False,
                   num_devices=N_CORES)

    widx_d = nc.declare_dram_parameter("widx", [P, NTILES], I32, isOutput=False)
    cidx_d = nc.declare_dram_parameter("cidx", [P, NTILES], I32, isOutput=False)
    meta_d = nc.declare_dram_parameter("meta", [P, 3 * NTILES], F32, isOutput=False)
    w_word_d = nc.declare_dram_parameter("W_word", [V, H], F8, isOutput=False)
    w_comb_d = nc.declare_dram_parameter("W_comb", [COMB, H], BF16, isOutput=False)
    lwq_d = nc.declare_dram_parameter("lwq", [3, P, 2 * H], F8, isOutput=False)
    lin7_d = nc.declare_dram_parameter("lin7", [65, H], F32, isOutput=False)
    wphi_d = nc.declare_dram_parameter("wphi", [1, 128], F32, isOutput=False)
    identf_d = nc.declare_dram_parameter("identf", [P, P], F32, isOutput=False)
    identu_d = nc.declare_dram_parameter("identu", [P, P], F8, isOutput=False)
    if apply_gb:
        ln_g_d = nc.declare_dram_parameter("ln_g", [H], F32, isOutput=False)
        ln_b_d = nc.declare_dram_parameter("ln_beta", [H], F32, isOutput=False)
    out_d = nc.declare_dram_parameter("out", [TOK, H], BF16, isOutput=True)

    with tile.TileContext(nc) as tc:
        with (
            tc.tile_pool(name="singles", bufs=1) as singles,
            tc.tile_pool(name="wgp", bufs=4) as wgp,
            tc.tile_pool(name="wtp", bufs=3) as wtp,
            tc.tile_pool(name="sp", bufs=3) as spool,
            tc.tile_pool(name="ep", bufs=GTILES + 4) as epool,
            tc.tile_pool(name="op", bufs=4) as opool,
            tc.tile_pool(name="vp", bufs=4) as vpool,
            tc.tile_pool(name="mvp", bufs=2) as mvpool,
            tc.tile_pool(name="tpu", bufs=1, space="PSUM") as tpup,
            tc.tile_pool(name="tps", bufs=2, space="PSUM") as tpsp,
            tc.tile_pool(name="mmp", bufs=2, space="PSUM") as mmp,
        ):
            # ---- constants / per-core staging ----
            lwq = singles.tile([P, 3, 2 * H], F8, tag="lwq")
            for q in range(3):
                nc.sync.dma_start(out=lwq[:, q, :], in_=lwq_d[q])
            l7stg = singles.tile([65, H], F32, tag="l7stg")
            nc.sync.dma_start(out=l7stg[:], in_=lin7_d[:])
            l7 = singles.tile([65, H], F32R, tag="l7")
            nc.vector.tensor_copy(out=l7[:], in_=l7stg[:])
            wp = singles.tile([P, 128], F32, tag="wp")
            nc.sync.dma_start(out=wp[:], in_=_bcast_rows(wphi_d[0]))
            mt = singles.tile([P, 3 * NTILES], F32, tag="mt")
            nc.sync.dma_start(out=mt[:], in_=meta_d[:])
            widx = singles.tile([P, NTILES], I32, tag="widx")
            nc.sync.dma_start(out=widx[:], in_=widx_d[:])
            cidx = singles.tile([P, NTILES], I32, tag="cidx")
            nc.sync.dma_start(out=cidx[:], in_=cidx_d[:])
            identf = singles.tile([P, P], F32, tag="identf")
            nc.sync.dma_start(out=identf[:], in_=identf_d[:])
            identu = singles.tile([P, P], F8, tag="identu")
            nc.sync.dma_start(out=identu[:], in_=identu_d[:])
            eps_sb = singles.tile([P, 1], F32, tag="eps")
            nc.vector.memset(eps_sb[:], EPS)
            # dt = ts - ts_prev for all 64 tiles at once
            dt = singles.tile([P, NTILES], F32, tag="dt")
            nc.vector.tensor_tensor(
                out=dt[:], in0=mt[:, 0:NTILES], in1=mt[:, NTILES:2 * NTILES],
                op=mybir.AluOpType.subtract,
            )
            # sin-part lhsT buffers; row 64 is the constant-1 bias row
            ones_sb = singles.tile([1, P], F32, tag="ones")
            nc.vector.memset(ones_sb[:], 1.0)
            # fp8 transpose targets: PE writes only even bytes (element
            # step 2); memset once so the u16-view copyback reads fully
            # initialized memory.  Manually double-buffered.
            tpu_bufs = []
            for j in range(2):
                t = tpup.tile([P, 2 * H], F8, tag=f"tpu{j}", space="PSUM")
                nc.vector.memset(t[:].bitcast(F32), 0.0)
                tpu_bufs.append(t)
            sT = []
            for j in range(3):
                t = singles.tile([65, P], F32R, tag=f"sT{j}")
                nc.vector.tensor_copy(out=t[64:65, :], in_=ones_sb[:])
                sT.append(t)
            if apply_gb:
                g_sb = singles.tile([P, H], BF16, tag="g")
                nc.gpsimd.dma_start(out=g_sb[:], in_=_bcast_rows(ln_g_d[:]))
                b_sb = singles.tile([P, H], BF16, tag="b")
                nc.gpsimd.dma_start(out=b_sb[:], in_=_bcast_rows(ln_b_d[:]))

            age0 = 2 * NTILES
            for g in range(NG):
                mvg = mvpool.tile([P, GTILES, 2], F32, tag="mvg")
                embs = []
                for ti in range(GTILES):
                    i = g * GTILES + ti
                    # ---- word gather (fp8, x16) ----
                    wg = wgp.tile([P, H], F8, tag="wg")
                    nc.gpsimd.indirect_dma_start(
                        out=wg[:], out_offset=None,
                        in_=w_word_d[:, :],
                        in_offset=IndirectOffsetOnAxis(
                            ap=widx[:, i:i + 1], axis=0),
                    )
                    # ---- transpose fp8 chunks (feature-major for lhsT) ----
                    # fp8 transposes write with element step 2 (HW rule)
                    tpu = tpu_bufs[i % 2]
                    tpu_ap = tpu[:]
                    for c in range(6):
                        nc.tensor.transpose(
                            out=bass.AP(
                                tensor=tpu_ap.tensor,
                                offset=tpu_ap.offset + 256 * c,
                                ap=[list(tpu_ap.ap[0]), [2, P]],
                            ),
                            in_=wg[:, c * P:(c + 1) * P], identity=identu[:],
                        )
                    wt = wtp.tile([P, 2 * H], F8, tag="wt")
                    nc.vector.tensor_copy(
                        out=wt[:].bitcast(U16), in_=tpu[:].bitcast(U16))

                    # ---- sin features (turns; exact wrap to [-.5,.5]) ----
                    s = spool.tile([P, 64], F32, tag="s")
                    nc.vector.scalar_tensor_tensor(
                        out=s[:, 0:T], in0=wp[:, 0:T],
                        scalar=dt[:, i:i + 1],
                        in1=wp[:, 64:64 + T],
                        op0=mybir.AluOpType.mult, op1=mybir.AluOpType.add,
                    )
                    nc.vector.scalar_tensor_tensor(
                        out=s[:, T:2 * T], in0=wp[:, T:2 * T],
                        scalar=mt[:, age0 + i:age0 + i + 1],
                        in1=wp[:, 96:96 + T],
                        op0=mybir.AluOpType.mult, op1=mybir.AluOpType.add,
                    )
                    rnd = spool.tile([P, 64], F32, tag="rnd")
                    nc.vector.tensor_scalar(
                        out=rnd[:], in0=s[:],
                        scalar1=12582912.0, scalar2=12582912.0,
                        op0=mybir.AluOpType.add, op1=mybir.AluOpType.subtract,
                    )
                    nc.vector.tensor_sub(out=s[:], in0=s[:], in1=rnd[:])
                    nc.scalar.activation(
                        out=s[:], in_=s[:],
                        func=mybir.ActivationFunctionType.Sin,
                        scale=float(2 * np.pi),
                    )
                    tps = tpsp.tile([P, P], F32, tag="tps", space="PSUM")
                    nc.tensor.transpose(
                        out=tps[0:64, :], in_=s[:], identity=identf[:])
                    st = sT[i % 3]
                    nc.scalar.copy(out=st[0:64, :], in_=tps[0:64, :])

                    # ---- matmul: 3 fp8 DoubleRow chunks + f32r sin chunk ----
                    mm = mmp.tile([P, H], F32, tag="mm", space="PSUM")
                    wt_f8 = wt[:]
                    for q in range(3):
                        lhsT = _pair_lhsT(wt_f8, q)
                        lwq_ap = lwq[:]
                        for n0, n1 in ((0, 512), (512, 768)):
                            nc.tensor.matmul(
                                out=mm[:, n0:n1], lhsT=lhsT,
                                rhs=bass.AP(
                                    tensor=lwq_ap.tensor,
                                    offset=lwq_ap.offset + q * 2 * H + n0,
                                    ap=[list(lwq_ap.ap[0]), [H, 2], [1, n1 - n0]],
                                ),
                                start=(q == 0), stop=False,
                                perf_mode=mybir.MatmulPerfMode.DoubleRow,
                            )
                    for n0, n1 in ((0, 512), (512, 768)):
                        nc.tensor.matmul(
                            out=mm[:, n0:n1], lhsT=st[:], rhs=l7[:, n0:n1],
                            start=False, stop=True,
                        )

                    # ---- tanh (undo x256 scale) -> bf16, then comb accum ----
                    emb = epool.tile([P, H], BF16, tag="emb")
                    nc.scalar.activation(
                        out=emb[:], in_=mm[:],
                        func=mybir.ActivationFunctionType.Tanh,
                        scale=float(1.0 / MMSCALE),
                    )
                    nc.gpsimd.indirect_dma_start(
                        out=emb[:], out_offset=None,
                        in_=w_comb_d[:, :],
                        in_offset=IndirectOffsetOnAxis(
                            ap=cidx[:, i:i + 1], axis=0),
                        compute_op=mybir.AluOpType.add,
                    )
                    embs.append(emb)

                    # ---- LN stats ----
                    stats = vpool.tile([P, 2, 6], F32, tag="st")
                    nc.vector.bn_stats(out=stats[:, 0, :], in_=emb[:, 0:384])
                    nc.vector.bn_stats(out=stats[:, 1, :], in_=emb[:, 384:768])
                    nc.vector.bn_aggr(out=mvg[:, ti, :], in_=stats[:])

                # ---- group: rstd then per-tile normalize ----
                mvg_ap = mvg[:]
                var_ap = bass.AP(
                    tensor=mvg_ap.tensor, offset=mvg_ap.offset + 1,
                    ap=[list(mvg_ap.ap[0]), [2, GTILES]],
                )
                sdg = vpool.tile([P, GTILES], F32, tag="sdg")
                nc.scalar.activation(
                    out=sdg[:], in_=var_ap,
                    func=mybir.ActivationFunctionType.Sqrt, bias=eps_sb[:],
                )
                rstdg = mvpool.tile([P, GTILES], F32, tag="rstdg")
                nc.vector.reciprocal(out=rstdg[:], in_=sdg[:])

                for ti in range(GTILES):
                    i = g * GTILES + ti
                    emb = embs[ti]
                    outt = opool.tile([P, H], BF16, tag="outt")
                    if ti % 2 == 0:
                        nc.vector.tensor_scalar(
                            out=outt[:], in0=emb[:],
                            scalar1=mvg[:, ti, 0:1], scalar2=rstdg[:, ti:ti + 1],
                            op0=mybir.AluOpType.subtract,
                            op1=mybir.AluOpType.mult,
                        )
                    else:
                        nmr = vpool.tile([P, 1], F32, tag="nmr")
                        nc.vector.tensor_scalar(
                            out=nmr[:], in0=mvg[:, ti, 0:1],
                            scalar1=rstdg[:, ti:ti + 1], scalar2=-1.0,
                            op0=mybir.AluOpType.mult, op1=mybir.AluOpType.mult,
                        )
                        nc.scalar.activation(
                            out=outt[:], in_=emb[:],
                            func=mybir.ActivationFunctionType.Identity,
                            bias=nmr[:], scale=rstdg[:, ti:ti + 1],
                        )
                    if apply_gb:
                        nc.vector.tensor_mul(out=outt[:], in0=outt[:], in1=g_sb[:])
                        nc.vector.tensor_add(out=outt[:], in0=outt[:], in1=b_sb[:])
                    nc.sync.dma_start(out=out_d[i * P:(i + 1) * P, :], in_=outt[:])

    nc.finalize()
    return nc


def _prepare(inputs):
    ids = np.asarray(inputs["input_ids"], dtype=np.int64)
    typ = np.asarray(inputs["type_ids"], dtype=np.int64)
    order = np.asarray(inputs["visit_orders"], dtype=np.int64)
    seg = np.asarray(inputs["visit_segments"], dtype=np.int64)
    ts = np.asarray(inputs["time_stamps"], dtype=np.float32)
    ages = np.asarray(inputs["ages"], dtype=np.float32)

    comb_idx = (order * (TYPES * SEGS) + typ * SEGS + seg).astype(np.int32)
    word_idx = ids.astype(np.int32)
    # halo: ts_prev[b, 0] = ts[b, 0] so dt[b, 0] == 0 (matches reference)
    ts_prev = np.concatenate([ts[:, :1], ts[:, :-1]], axis=1)

    f32c = lambda x: np.asarray(x, dtype=np.float32)
    w_type = f32c(inputs["W_type"])
    w_seg = f32c(inputs["W_seg"])
    w_order = f32c(inputs["W_order"])
    w_comb = (w_order[:, None, None, :] + w_type[None, :, None, :]
              + w_seg[None, None, :, :]).reshape(COMB, H).astype(BF16_NP)
    lin_w = f32c(inputs["lin_W"])
    # word part of lin_W: x16, fp8; DoubleRow pair q k-tile j covers rows
    # [256q+128j : 256q+128j+128] -> [3, 128, 2, 768] with (q, p, j, n)
    l0f8 = (lin_w[0:H] * WSCALE).astype(F8_NP)
    lwq = np.ascontiguousarray(
        l0f8.reshape(3, 2, P, H).transpose(0, 2, 1, 3).reshape(3, P, 2 * H))
    # sin part + bias row: x256 (the tanh un-scales by 1/256)
    lin7 = np.concatenate(
        [lin_w[H:H + 2 * T], f32c(inputs["lin_b"])[None, :]],
        axis=0) * np.float32(MMSCALE)
    # w/phi divided by 2pi: the kernel computes turns, wraps into
    # [-0.5, 0.5] via round-and-subtract, and Sin applies scale=2pi
    inv2pi = np.float64(1.0 / (2 * np.pi))
    wphi = np.concatenate(
        [f32c(inputs["time_w"])[0] * inv2pi, f32c(inputs["age_w"])[0] * inv2pi,
         f32c(inputs["time_phi"])[0] * inv2pi, f32c(inputs["age_phi"])[0] * inv2pi],
    ).astype(np.float32)[None, :]

    common = dict(
        W_word=np.ascontiguousarray(
            (f32c(inputs["W_word"]) * WSCALE).astype(F8_NP)),
        W_comb=np.ascontiguousarray(w_comb),
        lwq=lwq,
        lin7=np.ascontiguousarray(lin7.astype(np.float32)),
        wphi=np.ascontiguousarray(wphi),
        identf=np.eye(P, dtype=np.float32),
        identu=np.eye(P).astype(F8_NP),
    )

    ln_g = f32c(inputs["ln_g"])
    ln_beta = f32c(inputs["ln_beta"])
    apply_gb = not (np.all(ln_g == 1.0) and np.all(ln_beta == 0.0))
    if apply_gb:
        common["ln_g"] = ln_g
        common["ln_beta"] = ln_beta

    in_maps = []
    for k in range(N_CORES):
        sl = slice(k * B_PER, (k + 1) * B_PER)
        # token t -> partition t%128, column t//128
        tile_T = lambda a: np.ascontiguousarray(
            a[sl].reshape(TOK).reshape(NTILES, P).T)
        meta = np.concatenate(
            [tile_T(ts), tile_T(ts_prev), tile_T(ages)], axis=1)
        m = dict(common)
        m["widx"] = tile_T(word_idx)
        m["cidx"] = tile_T(comb_idx)
        m["meta"] = np.ascontiguousarray(meta)
        in_maps.append(m)
    return in_maps, apply_gb


def run(inputs, trace=False):
    in_maps, apply_gb = _prepare(inputs)
    nc = build_nc(apply_gb)
    res = run_bass_kernel_spmd(nc, in_maps, list(range(N_CORES)), trace=trace)
    shards = [res.results[k]["out"].astype(np.float32).reshape(B_PER, S, H)
              for k in range(N_CORES)]
    out = np.concatenate(shards, axis=0)
    return out, res


def kernel(**inputs) -> np.ndarray:
    out, _ = run(inputs, trace=False)
    return out


# revision 36
# speedup vs baseline: 1.7987x; 1.0844x over previous
"""Trainium2 Bass kernel for CachedEHREmbeddings (embedding_lookup).

Strategy (data-parallel over batch, 4 batch rows -> 8192 tokens per core):
  - word embeddings: per-tile indirect-DMA row gathers from an fp8e4 copy
    of W_word (values pre-scaled x16).  Gathered rows are transposed on
    the PE as uint16 PAIRS, which lands features in exactly the
    interleaved layout the fp8 DoubleRow matmul wants (2 K-tiles per
    pass, 2x PE throughput).
  - order/type/seg: fused on host into one bf16 table
    W_comb[o*27+t*3+s]; added into the post-tanh tile by an indirect
    DMA with compute_op=add (CCE accumulate) -- no extra DVE pass.
  - time/age sinusoidal features on-chip; w/phi are pre-divided by 2pi,
    wrapped into [-0.5,0.5] turns via the f32 magic-number round, and
    Sin runs with scale=2pi (exact range reduction, sim-clean).
  - main matmul: 3 fp8 DoubleRow chunks + 1 f32r sin chunk (includes the
    bias row, all x256) accumulate in PSUM; tanh on ScalarE with
    scale=1/256 -> bf16.
  - LayerNorm: DVE bn_stats/bn_aggr per tile; Sqrt batched per 16-tile
    group (keeps the ScalarE activation-table on the sin/tanh set);
    normalize alternates DVE/ScalarE to balance engines; bf16 output
    (host upcasts to f32).
"""

import sys

for _p in ("/opt/trn_rl_repo",):
    if _p not in sys.path:
        sys.path.insert(0, _p)

import ml_dtypes
import numpy as np

import concourse.bass as bass
import concourse.bacc as bacc
import concourse.tile as tile
from concourse import hw_specs, mybir
from concourse.bass import IndirectOffsetOnAxis
from concourse.bass_utils import run_bass_kernel_spmd

# Restrict the activation-table sets the load-placement pass may pick so
# Sin and Tanh resolve to the SAME set (silu_and_others) instead of
# toggling 1.3us ACT_TABLE_LOADs every tile.  Names keep their positions
# (act_func_set_id is the index into act_info.json's list); non-kept sets
# just become empty and are never chosen.
_ACT_SETS_KEEP = ("silu_and_others", "sqrt_and_others")
_orig_get_act_tables = hw_specs.get_activation_tables


def _restricted_act_tables(arch):
    full = _orig_get_act_tables(arch)
    return {name: (funcs if name in _ACT_SETS_KEEP else set())
            for name, funcs in full.items()}


bacc.get_activation_tables = _restricted_act_tables

# Problem constants (hardcoded per contract)
V, H, T = 32000, 768, 32
TYPES, MAX_VISITS, SEGS = 9, 512, 3
COMB = MAX_VISITS * TYPES * SEGS        # 13824 fused order/type/seg rows
B, S = 32, 2048
EPS = 1e-12
N_CORES = 8
B_PER = B // N_CORES                    # 4 batch rows per core
TOK = B_PER * S                         # 8192 tokens per core
P = 128
NTILES = TOK // P                       # 64
GTILES = 16                             # tiles per layernorm/sqrt group
NG = NTILES // GTILES

WSCALE = 16.0                           # fp8 scale for W_word and lin_W word part
MMSCALE = WSCALE * WSCALE               # total matmul scale (word path)

F32 = mybir.dt.float32
F32R = mybir.dt.float32r
BF16 = mybir.dt.bfloat16
F8 = mybir.dt.float8e4
U16 = mybir.dt.uint16
I32 = mybir.dt.int32

BF16_NP = ml_dtypes.bfloat16
F8_NP = ml_dtypes.float8_e4m3


def _bcast_rows(ap, p=P):
    """Partition-broadcast a [n]-shaped DRAM AP to [p, n] (stride-0 rows)."""
    return bass.AP(tensor=ap.tensor, offset=ap.offset, ap=[[0, p]] + list(ap.ap))


def _pair_lhsT(wt_f8, q):
    """lhsT AP [128, 2, 128] for DoubleRow chunk-pair q of the transposed
    word tile.  FP8 PE transposes write with element step 2; chunks 2q and
    2q+1 are interleaved byte-wise in bytes [512q : 512q+512) (chunk 2q at
    even bytes, 2q+1 at odd).  Partition p of chunk c holds feature
    128c+p; k-tile j of pair q is chunk 2q+j."""
    return bass.AP(
        tensor=wt_f8.tensor, offset=wt_f8.offset + 512 * q,
        ap=[list(wt_f8.ap[0]), [256, 2], [2, P]],
    )


def build_nc(apply_gb: bool):
    nc = bacc.Bacc("TRN2", target_bir_lowering=False, debug=False,
                   num_devices=N_CORES)

    widx_d = nc.declare_dram_parameter("widx", [P, NTILES], I32, isOutput=False)
    cidx_d = nc.declare_dram_parameter("cidx", [P, NTILES], I32, isOutput=False)
    meta_d = nc.declare_dram_parameter("meta", [P, 3 * NTILES], F32, isOutput=False)
    w_word_d = nc.declare_dram_parameter("W_word", [V, H], F8, isOutput=False)
    w_comb_d = nc.declare_dram_parameter("W_comb", [COMB, H], BF16, isOutput=False)
    lwq_d = nc.declare_dram_parameter("lwq", [3, P, 2 * H], F8, isOutput=False)
    lin7_d = nc.declare_dram_parameter("lin7", [65, H], F32, isOutput=False)
    wphi_d = nc.declare_dram_parameter("wphi", [1, 128], F32, isOutput=False)
    identf_d = nc.declare_dram_parameter("identf", [P, P], F32, isOutput=False)
    identu_d = nc.declare_dram_parameter("identu", [P, P], F8, isOutput=False)
    if apply_gb:
        ln_g_d = nc.declare_dram_parameter("ln_g", [H], F32, isOutput=False)
        ln_b_d = nc.declare_dram_parameter("ln_beta", [H], F32, isOutput=False)
    out_d = nc.declare_dram_parameter("out", [TOK, H], BF16, isOutput=True)

    with tile.TileContext(nc) as tc:
        with (
            tc.tile_pool(name="singles", bufs=1) as singles,
            tc.tile_pool(name="wgp", bufs=4) as wgp,
            tc.tile_pool(name="wtp", bufs=3) as wtp,
            tc.tile_pool(name="sp", bufs=3) as spool,
            tc.tile_pool(name="ep", bufs=GTILES + 4) as epool,
            tc.tile_pool(name="op", bufs=4) as opool,
            tc.tile_pool(name="vp", bufs=4) as vpool,
            tc.tile_pool(name="mvp", bufs=2) as mvpool,
            tc.tile_pool(name="tpu", bufs=1, space="PSUM") as tpup,
            tc.tile_pool(name="tps", bufs=2, space="PSUM") as tpsp,
            tc.tile_pool(name="mmp", bufs=2, space="PSUM") as mmp,
        ):
            # ---- constants / per-core staging ----
            lwq = singles.tile([P, 3, 2 * H], F8, tag="lwq")
            for q in range(3):
                nc.sync.dma_start(out=lwq[:, q, :], in_=lwq_d[q])
            l7stg = singles.tile([65, H], F32, tag="l7stg")
            nc.sync.dma_start(out=l7stg[:], in_=lin7_d[:])
            l7 = singles.tile([65, H], F32R, tag="l7")
            nc.vector.tensor_copy(out=l7[:], in_=l7stg[:])
            wp = singles.tile([P, 128], F32, tag="wp")
            nc.sync.dma_start(out=wp[:], in_=_bcast_rows(wphi_d[0]))
            mt = singles.tile([P, 3 * NTILES], F32, tag="mt")
            nc.sync.dma_start(out=mt[:], in_=meta_d[:])
            widx = singles.tile([P, NTILES], I32, tag="widx")
            nc.sync.dma_start(out=widx[:], in_=widx_d[:])
            cidx = singles.tile([P, NTILES], I32, tag="cidx")
            nc.sync.dma_start(out=cidx[:], in_=cidx_d[:])
            identf = singles.tile([P, P], F32, tag="identf")
            nc.sync.dma_start(out=identf[:], in_=identf_d[:])
            identu = singles.tile([P, P], F8, tag="identu")
            nc.sync.dma_start(out=identu[:], in_=identu_d[:])
            eps_sb = singles.tile([P, 1], F32, tag="eps")
            nc.vector.memset(eps_sb[:], EPS)
            # dt = ts - ts_prev for all 64 tiles at once
            dt = singles.tile([P, NTILES], F32, tag="dt")
            nc.vector.tensor_tensor(
                out=dt[:], in0=mt[:, 0:NTILES], in1=mt[:, NTILES:2 * NTILES],
                op=mybir.AluOpType.subtract,
            )
            # sin-part lhsT buffers; row 64 is the constant-1 bias row
            ones_sb = singles.tile([1, P], F32, tag="ones")
            nc.vector.memset(ones_sb[:], 1.0)
            # fp8 transpose targets: PE writes only even bytes (element
            # step 2); memset once so the u16-view copyback reads fully
            # initialized memory.  Manually double-buffered.
            tpu_bufs = []
            for j in range(2):
                t = tpup.tile([P, 2 * H], F8, tag=f"tpu{j}", space="PSUM")
                nc.vector.memset(t[:].bitcast(F32), 0.0)
                tpu_bufs.append(t)
            sT = []
            for j in range(3):
                t = singles.tile([65, P], F32R, tag=f"sT{j}")
                nc.vector.tensor_copy(out=t[64:65, :], in_=ones_sb[:])
                sT.append(t)
            if apply_gb:
                g_sb = singles.tile([P, H], BF16, tag="g")
                nc.gpsimd.dma_start(out=g_sb[:], in_=_bcast_rows(ln_g_d[:]))
                b_sb = singles.tile([P, H], BF16, tag="b")
                nc.gpsimd.dma_start(out=b_sb[:], in_=_bcast_rows(ln_b_d[:]))

            age0 = 2 * NTILES
            for g in range(NG):
                mvg = mvpool.tile([P, GTILES, 2], F32, tag="mvg")
                embs = []
                for ti in range(GTILES):
                    i = g * GTILES + ti
                    # ---- word gather (fp8, x16) ----
                    wg = wgp.tile([P, H], F8, tag="wg")
                    nc.gpsimd.indirect_dma_start(
                        out=wg[:], out_offset=None,
                        in_=w_word_d[:, :],
                        in_offset=IndirectOffsetOnAxis(
                            ap=widx[:, i:i + 1], axis=0),
                    )
                    # ---- transpose fp8 chunks (feature-major for lhsT) ----
                    # fp8 transposes write with element step 2 (HW rule)
                    tpu = tpu_bufs[i % 2]
                    tpu_ap = tpu[:]
                    for c in range(6):
                        nc.tensor.transpose(
                            out=bass.AP(
                                tensor=tpu_ap.tensor,
                                offset=tpu_ap.offset + 256 * c,
                                ap=[list(tpu_ap.ap[0]), [2, P]],
                            ),
                            in_=wg[:, c * P:(c + 1) * P], identity=identu[:],
                        )
                    wt = wtp.tile([P, 2 * H], F8, tag="wt")
                    nc.vector.tensor_copy(
                        out=wt[:].bitcast(U16), in_=tpu[:].bitcast(U16))

                    # ---- sin features (turns; exact wrap to [-.5,.5]) ----
                    s = spool.tile([P, 64], F32, tag="s")
                    nc.vector.scalar_tensor_tensor(
                        out=s[:, 0:T], in0=wp[:, 0:T],
                        scalar=dt[:, i:i + 1],
                        in1=wp[:, 64:64 + T],
                        op0=mybir.AluOpType.mult, op1=mybir.AluOpType.add,
                    )
                    nc.vector.scalar_tensor_tensor(
                        out=s[:, T:2 * T], in0=wp[:, T:2 * T],
                        scalar=mt[:, age0 + i:age0 + i + 1],
                        in1=wp[:, 96:96 + T],
                        op0=mybir.AluOpType.mult, op1=mybir.AluOpType.add,
                    )
                    rnd = spool.tile([P, 64], F32, tag="rnd")
                    nc.vector.tensor_scalar(
                        out=rnd[:], in0=s[:],
                        scalar1=12582912.0, scalar2=12582912.0,
                        op0=mybir.AluOpType.add, op1=mybir.AluOpType.subtract,
                    )
                    nc.vector.tensor_sub(out=s[:], in0=s[:], in1=rnd[:])
                    nc.scalar.activation(
                        out=s[:], in_=s[:],
                        func=mybir.ActivationFunctionType.Sin,
                        scale=float(2 * np.pi),
                    )
                    tps = tpsp.tile([P, P], F32, tag="tps", space="PSUM")
                    nc.tensor.transpose(
                        out=tps[0:64, :], in_=s[:], identity=identf[:])
                    st = sT[i % 3]
                    nc.scalar.copy(out=st[0:64, :], in_=tps[0:64, :])

                    # ---- matmul: 3 fp8 DoubleRow chunks + f32r sin chunk ----
                    mm = mmp.tile([P, H], F32, tag="mm", space="PSUM")
                    wt_f8 = wt[:]
                    for q in range(3):
                        lhsT = _pair_lhsT(wt_f8, q)
                        lwq_ap = lwq[:]
                        for n0, n1 in ((0, 512), (512, 768)):
                            nc.tensor.matmul(
                                out=mm[:, n0:n1], lhsT=lhsT,
                                rhs=bass.AP(
                                    tensor=lwq_ap.tensor,
                                    offset=lwq_ap.offset + q * 2 * H + n0,
                                    ap=[list(lwq_ap.ap[0]), [H, 2], [1, n1 - n0]],
                                ),
                                start=(q == 0), stop=False,
                                perf_mode=mybir.MatmulPerfMode.DoubleRow,
                            )
                    for n0, n1 in ((0, 512), (512, 768)):
                        nc.tensor.matmul(
                            out=mm[:, n0:n1], lhsT=st[:], rhs=l7[:, n0:n1],
                            start=False, stop=True,
                        )

                    # ---- tanh (undo x256 scale) -> bf16, then comb accum ----
                    emb = epool.tile([P, H], BF16, tag="emb")
                    nc.scalar.activation(
                        out=emb[:], in_=mm[:],
                        func=mybir.ActivationFunctionType.Tanh,
                        scale=float(1.0 / MMSCALE),
                    )
                    nc.gpsimd.indirect_dma_start(
                        out=emb[:], out_offset=None,
                        in_=w_comb_d[:, :],
                        in_offset=IndirectOffsetOnAxis(
                            ap=cidx[:, i:i + 1], axis=0),
                        compute_op=mybir.AluOpType.add,
                    )
                    embs.append(emb)

                    # ---- LN stats ----
                    stats = vpool.tile([P, 2, 6], F32, tag="st")
                    nc.vector.bn_stats(out=stats[:, 0, :], in_=emb[:, 0:384])
                    nc.vector.bn_stats(out=stats[:, 1, :], in_=emb[:, 384:768])
                    nc.vector.bn_aggr(out=mvg[:, ti, :], in_=stats[:])

                # ---- group: rstd then per-tile normalize ----
                mvg_ap = mvg[:]
                var_ap = bass.AP(
                    tensor=mvg_ap.tensor, offset=mvg_ap.offset + 1,
                    ap=[list(mvg_ap.ap[0]), [2, GTILES]],
                )
                sdg = vpool.tile([P, GTILES], F32, tag="sdg")
                nc.scalar.activation(
                    out=sdg[:], in_=var_ap,
                    func=mybir.ActivationFunctionType.Sqrt, bias=eps_sb[:],
                )
                rstdg = mvpool.tile([P, GTILES], F32, tag="rstdg")
                nc.vector.reciprocal(out=rstdg[:], in_=sdg[:])

                for ti in range(GTILES):
                    i = g * GTILES + ti
                    emb = embs[ti]
                    outt = opool.tile([P, H], BF16, tag="outt")
                    if ti % 4 != 3:
                        nc.vector.tensor_scalar(
                            out=outt[:], in0=emb[:],
                            scalar1=mvg[:, ti, 0:1], scalar2=rstdg[:, ti:ti + 1],
                            op0=mybir.AluOpType.subtract,
                            op1=mybir.AluOpType.mult,
                        )
                    else:
                        nmr = vpool.tile([P, 1], F32, tag="nmr")
                        nc.vector.tensor_scalar(
                            out=nmr[:], in0=mvg[:, ti, 0:1],
                            scalar1=rstdg[:, ti:ti + 1], scalar2=-1.0,
                            op0=mybir.AluOpType.mult, op1=mybir.AluOpType.mult,
                        )
                        nc.scalar.activation(
                            out=outt[:], in_=emb[:],
                            func=mybir.ActivationFunctionType.Identity,
                            bias=nmr[:], scale=rstdg[:, ti:ti + 1],
                        )
                    if apply_gb:
                        nc.vector.tensor_mul(out=outt[:], in0=outt[:], in1=g_sb[:])
                        nc.vector.tensor_add(out=outt[:], in0=outt[:], in1=b_sb[:])
                    nc.sync.dma_start(out=out_d[i * P:(i + 1) * P, :], in_=outt[:])

    nc.finalize()
    return nc


def _prepare(inputs):
    ids = np.asarray(inputs["input_ids"], dtype=np.int64)
    typ = np.asarray(inputs["type_ids"], dtype=np.int64)
    order = np.asarray(inputs["visit_orders"], dtype=np.int64)
    seg = np.asarray(inputs["visit_segments"], dtype=np.int64)
    ts = np.asarray(inputs["time_stamps"], dtype=np.float32)
    ages = np.asarray(inputs["ages"], dtype=np.float32)

    comb_idx = (order * (TYPES * SEGS) + typ * SEGS + seg).astype(np.int32)
    word_idx = ids.astype(np.int32)
    # halo: ts_prev[b, 0] = ts[b, 0] so dt[b, 0] == 0 (matches reference)
    ts_prev = np.concatenate([ts[:, :1], ts[:, :-1]], axis=1)

    f32c = lambda x: np.asarray(x, dtype=np.float32)
    w_type = f32c(inputs["W_type"])
    w_seg = f32c(inputs["W_seg"])
    w_order = f32c(inputs["W_order"])
    w_comb = (w_order[:, None, None, :] + w_type[None, :, None, :]
              + w_seg[None, None, :, :]).reshape(COMB, H).astype(BF16_NP)
    lin_w = f32c(inputs["lin_W"])
    # word part of lin_W: x16, fp8; DoubleRow pair q k-tile j covers rows
    # [256q+128j : 256q+128j+128] -> [3, 128, 2, 768] with (q, p, j, n)
    l0f8 = (lin_w[0:H] * WSCALE).astype(F8_NP)
    lwq = np.ascontiguousarray(
        l0f8.reshape(3, 2, P, H).transpose(0, 2, 1, 3).reshape(3, P, 2 * H))
    # sin part + bias row: x256 (the tanh un-scales by 1/256)
    lin7 = np.concatenate(
        [lin_w[H:H + 2 * T], f32c(inputs["lin_b"])[None, :]],
        axis=0) * np.float32(MMSCALE)
    # w/phi divided by 2pi: the kernel computes turns, wraps into
    # [-0.5, 0.5] via round-and-subtract, and Sin applies scale=2pi
    inv2pi = np.float64(1.0 / (2 * np.pi))
    wphi = np.concatenate(
        [f32c(inputs["time_w"])[0] * inv2pi, f32c(inputs["age_w"])[0] * inv2pi,
         f32c(inputs["time_phi"])[0] * inv2pi, f32c(inputs["age_phi"])[0] * inv2pi],
    ).astype(np.float32)[None, :]

    common = dict(
        W_word=np.ascontiguousarray(
            (f32c(inputs["W_word"]) * WSCALE).astype(F8_NP)),
        W_comb=np.ascontiguousarray(w_comb),
        lwq=lwq,
        lin7=np.ascontiguousarray(lin7.astype(np.float32)),
        wphi=np.ascontiguousarray(wphi),
        identf=np.eye(P, dtype=np.float32),
        identu=np.eye(P).astype(F8_NP),
    )

    ln_g = f32c(inputs["ln_g"])
    ln_beta = f32c(inputs["ln_beta"])
    apply_gb = not (np.all(ln_g == 1.0) and np.all(ln_beta == 0.0))
    if apply_gb:
        common["ln_g"] = ln_g
        common["ln_beta"] = ln_beta

    in_maps = []
    for k in range(N_CORES):
        sl = slice(k * B_PER, (k + 1) * B_PER)
        # token t -> partition t%128, column t//128
        tile_T = lambda a: np.ascontiguousarray(
            a[sl].reshape(TOK).reshape(NTILES, P).T)
        meta = np.concatenate(
            [tile_T(ts), tile_T(ts_prev), tile_T(ages)], axis=1)
        m = dict(common)
        m["widx"] = tile_T(word_idx)
        m["cidx"] = tile_T(comb_idx)
        m["meta"] = np.ascontiguousarray(meta)
        in_maps.append(m)
    return in_maps, apply_gb


def run(inputs, trace=False):
    in_maps, apply_gb = _prepare(inputs)
    nc = build_nc(apply_gb)
    res = run_bass_kernel_spmd(nc, in_maps, list(range(N_CORES)), trace=trace)
    shards = [res.results[k]["out"].astype(np.float32).reshape(B_PER, S, H)
              for k in range(N_CORES)]
    out = np.concatenate(shards, axis=0)
    return out, res


def kernel(**inputs) -> np.ndarray:
    out, _ = run(inputs, trace=False)
    return out
